# revision 6
# baseline (speedup 1.0000x reference)
"""AttentionBlock (b=2, c=512, 64x64) on 8 trn2 NeuronCores.

Sharding: core i handles batch i//4, query rows (i%4)*1024..+1024 (of the
4096 flattened h*w positions). Each core receives its batch's full x with
columns rotated so its own query block sits at columns 0:1024, computes
LayerNorm stats + K + V for all 4096 positions (replicated inside the
4-core batch group) and Q/attention/projection for its 1024 queries.

Key structural points (v2):
  - x8 = fp8(x) straight away; the QKV matmuls do NOT wait on the
    LayerNorm stats.  The rsqrt(var+eps) factor r is applied after the
    matmuls instead:
      * K is evicted UNscaled (k8 = fp8(kp/sqrt(C))); the per-key factor
        r_k is folded into the softmax exp as a per-partition ACT scale
        (st has keys on partitions).
      * V is scaled at eviction by a per-partition scalar r_n.
      * Q is scaled by a broadcast row of r (its own 1024 positions only)
        plus the folded bias.
  - k-bias drops out of softmax entirely; v-bias is folded into the
    projection bias on the host; q-bias is kept (folded scale).
  - Attention AV matmuls run with V stationary, so the attention output
    lands directly in [c, q] layout: no PE transposes, and the
    projection consumes it as the moving operand straight away.
  - sumexp accumulates in PSUM across all 16 key-pair steps.
  - PSUM budget (8 banks): psA(2) stats ping/pong then sumexp;
    psB(4) kp/vp/qp rotation then the 4 avps accumulators;
    psC(2) st ping/pong then proj outputs.
"""
import sys

if "/opt/trn_rl_repo" not in sys.path:
    sys.path.insert(0, "/opt/trn_rl_repo")

import numpy as np

C = 512          # channels
N = 4096         # h*w positions
NQ = 1024        # queries per core
PC = 4           # c chunks of 128
NKC = 32         # key chunks of 128
NJ = 8           # x column chunks of 512
CH = 512         # x chunk width
EPS = 1e-5
SQC = 22.627416997969522   # sqrt(512)
Q8S = 32.0                 # q8 = 32/sqrt(C) * logit-ready q
EXPB = -1.5                # exp(logits + EXPB), cancels in softmax
MAGIC = 0x5F3759DF         # Quake rsqrt seed

_cached_nc = None


def _build_nc():
    import concourse.bass as bass
    import concourse.tile as tile
    from concourse import bacc, mybir

    f32 = mybir.dt.float32
    f32r = mybir.dt.float32r
    i32 = mybir.dt.int32
    f8 = mybir.dt.float8e4
    AF = mybir.ActivationFunctionType
    ALU = mybir.AluOpType
    DR = mybir.MatmulPerfMode.DoubleRow

    nc = bacc.Bacc(None, target_bir_lowering=False)

    xd = nc.declare_dram_parameter("x", [NJ, 128, PC, CH], f32r, isOutput=False)
    wqd = nc.declare_dram_parameter("wq", [128, PC, C], f8, isOutput=False)
    wkd = nc.declare_dram_parameter("wk", [128, PC, C], f8, isOutput=False)
    wvd = nc.declare_dram_parameter("wv", [128, PC, C], f8, isOutput=False)
    wpd = nc.declare_dram_parameter("wp", [128, PC, C], f8, isOutput=False)
    bqd = nc.declare_dram_parameter("bq", [128, PC], f32, isOutput=False)
    bpd = nc.declare_dram_parameter("bp", [128, PC], f32, isOutput=False)
    outd = nc.declare_dram_parameter("out", [C, NQ], f32, isOutput=True)

    outr = outd.rearrange("(a p) n -> p a n", p=128)   # [128, 4, NQ]

    with tile.TileContext(nc) as tc:
        from contextlib import ExitStack

        with ExitStack() as ctx:
            consts = ctx.enter_context(tc.tile_pool(name="consts", bufs=1))
            xpool = ctx.enter_context(tc.tile_pool(name="xpool", bufs=1))
            kvq = ctx.enter_context(tc.tile_pool(name="kvq", bufs=1))
            dramp = ctx.enter_context(
                tc.tile_pool(name="dramp", bufs=1, space="DRAM")
            )
            # PSUM: exactly 8 banks
            psA = ctx.enter_context(
                tc.tile_pool(name="psA", bufs=2, space=bass.MemorySpace.PSUM)
            )
            psB = ctx.enter_context(
                tc.tile_pool(name="psB", bufs=4, space=bass.MemorySpace.PSUM)
            )
            psC = ctx.enter_context(
                tc.tile_pool(name="psC", bufs=2, space=bass.MemorySpace.PSUM)
            )
            stage = ctx.enter_context(tc.tile_pool(name="stage", bufs=2))
            x2p = ctx.enter_context(tc.tile_pool(name="x2p", bufs=2))
            ptp = ctx.enter_context(tc.tile_pool(name="ptp", bufs=3))
            avn_pool = ctx.enter_context(tc.tile_pool(name="avn", bufs=2))
            out_pool = ctx.enter_context(tc.tile_pool(name="outp", bufs=1))
            small = ctx.enter_context(tc.tile_pool(name="small", bufs=2))

            ones2 = consts.tile([128, 2, 128], f8)
            nc.vector.memset(ones2, 1.0)
            ones_col = consts.tile([128, 1], f32r)
            nc.vector.memset(ones_col.bitcast(f32), 1.0)
            magict = consts.tile([4, 256], i32)
            nc.vector.memset(magict, MAGIC)
            expb = consts.tile([128, 1], f32)
            nc.vector.memset(expb, EXPB)

            bq_sb = consts.tile([128, PC], f32)
            bp_sb = consts.tile([128, PC], f32)
            wq_sb = consts.tile([128, PC, C], f8)
            wk_sb = consts.tile([128, PC, C], f8)
            wv_sb = consts.tile([128, PC, C], f8)
            wp_sb = consts.tile([128, PC, C], f8)

            x_sb = xpool.tile([128, PC, N], f32r)
            x8 = kvq.tile([128, PC, N], f8)
            k_all = kvq.tile([128, PC, N], f8)     # (c, n) layout
            v_all = kvq.tile([128, NKC, C], f8)    # (n, c) layout
            q_all = kvq.tile([128, PC, NQ], f8)    # (c, nq) layout
            rc_v = kvq.tile([128, NKC], f32)       # r/sqrt(C), keyed [p, chunk]
            rc_q = kvq.tile([128, NKC], f32)       # r/Q8S, keyed [p, chunk]
            rr = kvq.tile([128, NQ], f32)          # r broadcast, own queries

            r_dram = dramp.tile([1, N], f32)

            dmaeng = [nc.sync, nc.scalar, nc.gpsimd]

            # ---- phase 1 helpers ----
            def stats_chunk(j, stg_row):
                """Column sums of x and x^2 for 512-col chunk j."""
                xv = x_sb[:, :, j * CH:(j + 1) * CH]
                xsq = x2p.tile([128, PC, CH], f32r, name="xsq")
                nc.gpsimd.tensor_mul(xsq, xv, xv)
                ps_s = psA.tile([1, CH], f32, tag="a", name="ps_s")
                ps_q = psA.tile([1, CH], f32, tag="a", name="ps_q")
                for ci in range(PC):
                    nc.tensor.matmul(
                        ps_s, ones_col, xv[:, ci, :],
                        start=(ci == 0), stop=(ci == PC - 1),
                    )
                for ci in range(PC):
                    nc.tensor.matmul(
                        ps_q, ones_col, xsq[:, ci, :],
                        start=(ci == 0), stop=(ci == PC - 1),
                    )
                h = (j % 2) * CH
                nc.scalar.activation(stg_row[0:1, h:h + CH], ps_s, AF.Copy)
                nc.scalar.activation(
                    stg_row[0:1, 1024 + h:1024 + h + CH], ps_q, AF.Copy
                )

            def rchain(qb, stg_row):
                """r = rsqrt(var+eps) for quarter qb -> r_dram + rc/rr tiles.
                Quake rsqrt + 1 Newton step, DVE only."""
                sm = stage.tile([4, 256], f32, name="stgs", tag="stgs")
                sq = stage.tile([4, 256], f32, name="stgq", tag="stgq")
                nc.sync.dma_start(out=sm, in_=stg_row[0:1, 0:1024])
                nc.sync.dma_start(out=sq, in_=stg_row[0:1, 1024:2048])
                u2 = stage.tile([4, 256], f32, name="u2", tag="u2")
                nc.vector.tensor_mul(u2, sm, sm)
                z = stage.tile([4, 256], f32, name="z", tag="z")
                nc.vector.scalar_tensor_tensor(
                    out=z, in0=u2, scalar=-1.0 / C, in1=sq,
                    op0=ALU.mult, op1=ALU.add,
                )
                nc.vector.tensor_scalar_add(z, z, C * EPS)
                r0i = stage.tile([4, 256], i32, name="r0i", tag="r0i")
                nc.vector.tensor_scalar(
                    out=r0i, in0=z.bitcast(i32), scalar1=1, scalar2=None,
                    op0=ALU.logical_shift_right,
                )
                nc.vector.tensor_sub(r0i, magict, r0i)
                r0 = r0i.bitcast(f32)
                a2 = stage.tile([4, 256], f32, name="a2", tag="a2")
                nc.vector.tensor_mul(a2, r0, r0)
                nc.vector.tensor_mul(a2, a2, z)
                nc.vector.tensor_scalar(
                    out=a2, in0=a2, scalar1=-0.5 * SQC, scalar2=1.5 * SQC,
                    op0=ALU.mult, op1=ALU.add,
                )
                rt = stage.tile([4, 256], f32, name="rt", tag="rt")
                nc.vector.tensor_mul(rt, r0, a2)
                # rt = rsqrt(var+eps) for positions qb*1024..+1024
                nc.scalar.dma_start(
                    out=r_dram[0:1, qb * 1024:(qb + 1) * 1024], in_=rt
                )
                # per-partition key layout: rc[p, c8] = r[c8*128 + p]
                rg = r_dram[0:1, qb * 1024:(qb + 1) * 1024].rearrange(
                    "o (c p) -> p (o c)", p=128
                )
                rcw = stage.tile([128, 8], f32, name="rcw", tag="rcw")
                nc.sync.dma_start(out=rcw, in_=rg)
                nc.vector.tensor_scalar_mul(
                    rc_v[:, qb * 8:(qb + 1) * 8], rcw, 1.0 / SQC
                )
                nc.vector.tensor_scalar_mul(
                    rc_q[:, qb * 8:(qb + 1) * 8], rcw, 1.0 / Q8S
                )
                if qb == 0:
                    nc.gpsimd.dma_start(
                        out=rr,
                        in_=r_dram[0:1, 0:NQ].to_broadcast([128, NQ]),
                    )

            def x8_chunk(j):
                xv = x_sb[:, :, j * CH:(j + 1) * CH]
                for h in range(2):
                    eng = nc.gpsimd if h == 0 else nc.vector
                    eng.tensor_scalar_mul(
                        x8[:, 2 * h:2 * h + 2, j * CH:(j + 1) * CH],
                        xv[:, 2 * h:2 * h + 2, :], 1.0,
                    )

            def k_chunk(j):
                """K for 512-col chunk j: k8 = fp8(kp/sqrt(C)) (no r)."""
                for co in range(PC):
                    kp = psB.tile([128, CH], f32, tag="b", name="kp")
                    for i2 in range(2):
                        nc.tensor.matmul(
                            kp,
                            wk_sb[:, 2 * i2:2 * i2 + 2, co * 128:(co + 1) * 128],
                            x8[:, 2 * i2:2 * i2 + 2, j * CH:(j + 1) * CH],
                            start=(i2 == 0), stop=(i2 == 1), perf_mode=DR,
                        )
                    nc.scalar.activation(
                        k_all[:, co, j * CH:(j + 1) * CH], kp,
                        AF.Copy, scale=1.0 / SQC,
                    )

            def v_chunk(j):
                """V for chunk j: v8 = fp8(r_n/sqrt(C) * vp), [n, c] layout."""
                for s4 in range(4):
                    jk = 4 * j + s4
                    vp = psB.tile([128, C], f32, tag="b", name="vp")
                    for i2 in range(2):
                        nc.tensor.matmul(
                            vp,
                            x8[:, 2 * i2:2 * i2 + 2, jk * 128:(jk + 1) * 128],
                            wv_sb[:, 2 * i2:2 * i2 + 2, :],
                            start=(i2 == 0), stop=(i2 == 1), perf_mode=DR,
                        )
                    nc.vector.tensor_scalar(
                        out=v_all[:, jk, :], in0=vp,
                        scalar1=rc_v[:, jk:jk + 1], scalar2=None,
                        op0=ALU.mult,
                    )

            def q_chunk(j):
                """Q for own 512-col chunk j (j in {0,1}), with r and bias."""
                for co in range(PC):
                    qp = psB.tile([128, CH], f32, tag="b", name="qp")
                    for i2 in range(2):
                        nc.tensor.matmul(
                            qp,
                            wq_sb[:, 2 * i2:2 * i2 + 2, co * 128:(co + 1) * 128],
                            x8[:, 2 * i2:2 * i2 + 2, j * CH:(j + 1) * CH],
                            start=(i2 == 0), stop=(i2 == 1), perf_mode=DR,
                        )
                    qt = x2p.tile([128, CH], f32, tag="qt", name="qt")
                    nc.vector.scalar_tensor_tensor(
                        out=qt, in0=qp, scalar=Q8S / C,
                        in1=rr[:, j * CH:(j + 1) * CH],
                        op0=ALU.mult, op1=ALU.mult,
                    )
                    nc.vector.tensor_scalar(
                        out=q_all[:, co, j * CH:(j + 1) * CH], in0=qt,
                        scalar1=1.0, scalar2=bq_sb[:, co:co + 1],
                        op0=ALU.mult, op1=ALU.add,
                    )

            # ---- attention ----
            def attention_group(g):
                q0 = g * 512
                avps = [
                    psB.tile([128, 512], f32, tag="b", name=f"avp{g}{s}")
                    for s in range(PC)
                ]
                sp = psA.tile([128, 512], f32, tag="a", name=f"sp{g}")
                for pr in range(16):
                    pt2 = ptp.tile([128, 2, 512], f8, tag="pt", name="pt2")
                    for u in range(2):
                        jk = 2 * pr + u
                        st = psC.tile([128, 512], f32, tag="c", name="st")
                        for i2 in range(2):
                            nc.tensor.matmul(
                                st,
                                k_all[:, 2 * i2:2 * i2 + 2,
                                      jk * 128:(jk + 1) * 128],
                                q_all[:, 2 * i2:2 * i2 + 2, q0:q0 + 512],
                                start=(i2 == 0), stop=(i2 == 1), perf_mode=DR,
                            )
                        nc.scalar.activation(
                            pt2[:, u, :], st, AF.Exp,
                            scale=rc_q[:, jk:jk + 1], bias=expb,
                        )
                    for ci in range(PC):
                        nc.tensor.matmul(
                            avps[ci],
                            v_all[:, 2 * pr:2 * pr + 2, ci * 128:(ci + 1) * 128],
                            pt2,
                            start=(pr == 0), stop=(pr == 15), perf_mode=DR,
                        )
                    nc.tensor.matmul(
                        sp, ones2, pt2,
                        start=(pr == 0), stop=(pr == 15), perf_mode=DR,
                    )
                # softmax normalize + project + residual
                # (every partition of sp carries the same sumexp row)
                rcb = small.tile([128, 512], f32, tag="rcb", name=f"rcb{g}")
                nc.vector.reciprocal(rcb, sp)
                avn = avn_pool.tile([128, PC, 512], f8, name="avn")
                for ci in range(PC):
                    nc.vector.tensor_mul(avn[:, ci, :], avps[ci], rcb)
                out_t = out_pool.tile([128, PC, 512], f32, name="outt")
                for co in range(PC):
                    pop = psC.tile([128, 512], f32, tag="c", name="pop")
                    for i2 in range(2):
                        nc.tensor.matmul(
                            pop,
                            wp_sb[:, 2 * i2:2 * i2 + 2, co * 128:(co + 1) * 128],
                            avn[:, 2 * i2:2 * i2 + 2, :],
                            start=(i2 == 0), stop=(i2 == 1), perf_mode=DR,
                        )
                    nc.vector.tensor_scalar(
                        out=out_t[:, co, :], in0=pop,
                        scalar1=1.0 / SQC, scalar2=bp_sb[:, co:co + 1],
                        op0=ALU.mult, op1=ALU.add,
                    )
                    for h in range(2):
                        eng = nc.gpsimd if h == 0 else nc.vector
                        eng.tensor_add(
                            out_t[:, co, h * 256:(h + 1) * 256],
                            out_t[:, co, h * 256:(h + 1) * 256],
                            x_sb[:, co, q0 + h * 256:q0 + (h + 1) * 256],
                        )
                        (nc.sync if h == 0 else nc.scalar).dma_start(
                            out=outr[:, co, q0 + h * 256:q0 + (h + 1) * 256],
                            in_=out_t[:, co, h * 256:(h + 1) * 256],
                        )

            # ---- emission ----
            nc.scalar.dma_start(out=bq_sb, in_=bqd[:])
            nc.sync.dma_start(out=bp_sb, in_=bpd[:])
            nc.sync.dma_start(out=wk_sb, in_=wkd[:])
            nc.scalar.dma_start(out=wv_sb, in_=wvd[:])
            nc.scalar.dma_start(out=wq_sb, in_=wqd[:])
            nc.sync.dma_start(out=wp_sb, in_=wpd[:])
            for j in range(3):
                dmaeng[j % 2].dma_start(
                    out=x_sb[:, :, j * CH:(j + 1) * CH], in_=xd[j]
                )

            stg_rows = {}
            for j in range(NJ):
                if j + 3 < NJ:
                    dmaeng[(j + 3) % 2].dma_start(
                        out=x_sb[:, :, (j + 3) * CH:(j + 4) * CH],
                        in_=xd[j + 3],
                    )
                qb = j // 2
                if j % 2 == 0:
                    stg_rows[qb] = stage.tile(
                        [1, 2048], f32, name="sr", tag="sr"
                    )
                stats_chunk(j, stg_rows[qb])
                x8_chunk(j)
                k_chunk(j)
                if j % 2 == 1:
                    rchain(qb, stg_rows[qb])
                if j >= 2:
                    v_chunk(j - 2)
                if j - 2 in (0, 1):
                    q_chunk(j - 2)
            v_chunk(6)
            v_chunk(7)
            attention_group(0)
            attention_group(1)

    nc.compile()
    return nc


def _get_nc():
    global _cached_nc
    if _cached_nc is None:
        _cached_nc = _build_nc()
    return _cached_nc


def kernel(x, norm_w, w_qkv, b_qkv, w_proj, b_proj):
    import ml_dtypes

    f8np = ml_dtypes.float8_e4m3

    x = np.asarray(x, dtype=np.float32)
    norm_w = np.asarray(norm_w, dtype=np.float32)
    w_qkv = np.asarray(w_qkv, dtype=np.float32)
    b_qkv = np.asarray(b_qkv, dtype=np.float32)
    w_proj = np.asarray(w_proj, dtype=np.float32)
    b_proj = np.asarray(b_proj, dtype=np.float32)

    B = x.shape[0]

    # fold norm_w + LN centering into the QKV weights; sqrt(C) into all
    Wq = w_qkv[0:C] * norm_w[None, :]
    Wk = w_qkv[C:2 * C] * norm_w[None, :]
    Wv = w_qkv[2 * C:3 * C] * norm_w[None, :]

    def wtile(wt):  # [cin, cout] -> [128, PC, cout]
        return np.ascontiguousarray(
            wt.reshape(PC, 128, C).transpose(1, 0, 2).astype(f8np)
        )

    Wqt = wtile(((Wq - Wq.mean(1, keepdims=True)) * SQC).T)
    Wkt = wtile(((Wk - Wk.mean(1, keepdims=True)) * SQC).T)
    Wvt = wtile(((Wv - Wv.mean(1, keepdims=True)) * SQC).T)
    Wpt = wtile(w_proj.T * SQC)

    def cols(b):  # [C] -> [128, 4] chunk-column layout
        return np.ascontiguousarray(b.reshape(PC, 128).T)

    bq = cols(b_qkv[0:C] * (Q8S / SQC))
    bv = b_qkv[2 * C:3 * C]
    bpt = cols(b_proj + w_proj @ bv)

    in_maps = []
    for core in range(8):
        bi, qi = core // 4, core % 4
        xl = np.roll(x[bi].reshape(C, N), -qi * NQ, axis=1)
        # pre-tile to the on-chip layout: [chunk, partition, c-chunk, col]
        xl = np.ascontiguousarray(
            xl.reshape(PC, 128, NJ, CH).transpose(2, 1, 0, 3)
        )
        in_maps.append({
            "x": xl, "wq": Wqt, "wk": Wkt, "wv": Wvt, "wp": Wpt,
            "bq": bq, "bp": bpt,
        })

    from concourse.bass_utils import run_bass_kernel_spmd

    nc = _get_nc()
    res = run_bass_kernel_spmd(nc, in_maps, core_ids=list(range(8)))

    out = np.empty((B, C, N), dtype=np.float32)
    for core in range(8):
        bi, qi = core // 4, core % 4
        out[bi][:, qi * NQ:(qi + 1) * NQ] = res.results[core]["out"]
    return out.reshape(x.shape)


# revision 9
# speedup vs baseline: 1.5443x; 1.5443x over previous
"""AttentionBlock (b=2, c=512, 64x64) on 8 trn2 NeuronCores.

Sharding: core i handles batch i//4, query rows (i%4)*1024..+1024 (of the
4096 flattened h*w positions). Each core receives its batch's full x with
columns rotated so its own query block sits at columns 0:1024, computes
LayerNorm stats + K + V for all 4096 positions (replicated inside the
4-core batch group) and Q/attention/projection for its 1024 queries.

Key structural points (v2):
  - x8 = fp8(x) straight away; the QKV matmuls do NOT wait on the
    LayerNorm stats.  The rsqrt(var+eps) factor r is applied after the
    matmuls instead:
      * K is evicted UNscaled (k8 = fp8(kp/sqrt(C))); the per-key factor
        r_k is folded into the softmax exp as a per-partition ACT scale
        (st has keys on partitions).
      * V is scaled at eviction by a per-partition scalar r_n.
      * Q is scaled by a broadcast row of r (its own 1024 positions only)
        plus the folded bias.
  - k-bias drops out of softmax entirely; v-bias is folded into the
    projection bias on the host; q-bias is kept (folded scale).
  - Attention AV matmuls run with V stationary, so the attention output
    lands directly in [c, q] layout: no PE transposes, and the
    projection consumes it as the moving operand straight away.
  - sumexp accumulates in PSUM across all 16 key-pair steps.
  - PSUM budget (8 banks): psA(2) stats ping/pong then sumexp;
    psB(4) kp/vp/qp rotation then the 4 avps accumulators;
    psC(2) st ping/pong then proj outputs.
"""
import sys

if "/opt/trn_rl_repo" not in sys.path:
    sys.path.insert(0, "/opt/trn_rl_repo")

import numpy as np

C = 512          # channels
N = 4096         # h*w positions
NQ = 1024        # queries per core
PC = 4           # c chunks of 128
NKC = 32         # key chunks of 128
NJ = 8           # x column chunks of 512
CH = 512         # x chunk width
EPS = 1e-5
SQC = 22.627416997969522   # sqrt(512)
Q8S = 32.0                 # q8 = 32/sqrt(C) * logit-ready q
EXPB = -1.5                # exp(logits + EXPB), cancels in softmax
MAGIC = 0x5F3759DF         # Quake rsqrt seed

_cached_nc = None


def _build_nc():
    import concourse.bass as bass
    import concourse.tile as tile
    from concourse import bacc, mybir

    f32 = mybir.dt.float32
    f32r = mybir.dt.float32r
    i32 = mybir.dt.int32
    f8 = mybir.dt.float8e4
    AF = mybir.ActivationFunctionType
    ALU = mybir.AluOpType
    DR = mybir.MatmulPerfMode.DoubleRow

    nc = bacc.Bacc(None, target_bir_lowering=False)

    xd = nc.declare_dram_parameter("x", [NJ, 128, PC, CH], f32, isOutput=False)
    x8d = nc.declare_dram_parameter("x8", [NJ, 128, PC, CH], f8, isOutput=False)
    xqd = nc.declare_dram_parameter("xq8", [NJ, 128, PC, CH], f8, isOutput=False)
    wqd = nc.declare_dram_parameter("wq", [128, PC, C], f8, isOutput=False)
    wkd = nc.declare_dram_parameter("wk", [128, PC, C], f8, isOutput=False)
    wvd = nc.declare_dram_parameter("wv", [128, PC, C], f8, isOutput=False)
    wpd = nc.declare_dram_parameter("wp", [128, PC, C], f8, isOutput=False)
    bqd = nc.declare_dram_parameter("bq", [128, PC], f32, isOutput=False)
    bpd = nc.declare_dram_parameter("bp", [128, PC], f32, isOutput=False)
    outd = nc.declare_dram_parameter("out", [C, NQ], f32, isOutput=True)

    outr = outd.rearrange("(a p) n -> p a n", p=128)   # [128, 4, NQ]

    with tile.TileContext(nc) as tc:
        from contextlib import ExitStack

        with ExitStack() as ctx:
            consts = ctx.enter_context(tc.tile_pool(name="consts", bufs=1))
            xpool = ctx.enter_context(tc.tile_pool(name="xpool", bufs=1))
            kvq = ctx.enter_context(tc.tile_pool(name="kvq", bufs=1))
            dramp = ctx.enter_context(
                tc.tile_pool(name="dramp", bufs=1, space="DRAM")
            )
            # PSUM: exactly 8 banks
            psA = ctx.enter_context(
                tc.tile_pool(name="psA", bufs=2, space=bass.MemorySpace.PSUM)
            )
            psB = ctx.enter_context(
                tc.tile_pool(name="psB", bufs=4, space=bass.MemorySpace.PSUM)
            )
            psC = ctx.enter_context(
                tc.tile_pool(name="psC", bufs=2, space=bass.MemorySpace.PSUM)
            )
            stage = ctx.enter_context(tc.tile_pool(name="stage", bufs=2))
            x2p = ctx.enter_context(tc.tile_pool(name="x2p", bufs=2))
            ptp = ctx.enter_context(tc.tile_pool(name="ptp", bufs=3))
            avn_pool = ctx.enter_context(tc.tile_pool(name="avn", bufs=2))
            out_pool = ctx.enter_context(tc.tile_pool(name="outp", bufs=1))
            small = ctx.enter_context(tc.tile_pool(name="small", bufs=2))

            ones2 = consts.tile([128, 2, 128], f8)
            nc.vector.memset(ones2, 1.0)
            magict = consts.tile([4, 256], i32)
            nc.vector.memset(magict, MAGIC)
            expb = consts.tile([128, 1], f32)
            nc.vector.memset(expb, EXPB)

            bq_sb = consts.tile([128, PC], f32)
            bp_sb = consts.tile([128, PC], f32)
            wq_sb = consts.tile([128, PC, C], f8)
            wk_sb = consts.tile([128, PC, C], f8)
            wv_sb = consts.tile([128, PC, C], f8)
            wp_sb = consts.tile([128, PC, C], f8)

            x_sb = xpool.tile([128, PC, N], f32)
            x8 = kvq.tile([128, PC, N], f8)
            k_all = kvq.tile([128, PC, N], f8)     # (c, n) layout
            v_all = kvq.tile([128, NKC, C], f8)    # (n, c) layout
            q_all = kvq.tile([128, PC, NQ], f8)    # (c, nq) layout
            rc_v = kvq.tile([128, NKC], f32)       # r/sqrt(C), keyed [p, chunk]
            rc_q = kvq.tile([128, NKC], f32)       # r/Q8S, keyed [p, chunk]
            rr = kvq.tile([128, NQ], f32)          # r broadcast, own queries

            r_dram = dramp.tile([1, N], f32)

            dmaeng = [nc.sync, nc.scalar, nc.gpsimd]

            # ---- phase 1 helpers ----
            def stats_chunk(j, srow, x8q):
                """Column sums of x8 and x8^2 for 512-col chunk j (fp8 DR)."""
                ps_s = psA.tile([128, CH], f32, tag="a", name="ps_s")
                ps_q = psA.tile([128, CH], f32, tag="a", name="ps_q")
                for i2 in range(2):
                    nc.tensor.matmul(
                        ps_s, ones2,
                        x8[:, 2 * i2:2 * i2 + 2, j * CH:(j + 1) * CH],
                        start=(i2 == 0), stop=(i2 == 1), perf_mode=DR,
                    )
                for i2 in range(2):
                    nc.tensor.matmul(
                        ps_q, ones2, x8q[:, 2 * i2:2 * i2 + 2, :],
                        start=(i2 == 0), stop=(i2 == 1), perf_mode=DR,
                    )
                h = CH * (j % 2)
                nc.scalar.activation(srow[0:1, h:h + CH], ps_s[0:1, :], AF.Copy)
                nc.scalar.activation(
                    srow[0:1, 1024 + h:1024 + h + CH], ps_q[0:1, :], AF.Copy
                )

            def rchain(qb, srow):
                """r = rsqrt(var+eps) for quarter qb -> r_dram + rc/rr tiles.
                Quake rsqrt + 1 Newton step, DVE only."""
                sm = stage.tile([4, 256], f32, name="stgs", tag="stgs")
                sq = stage.tile([4, 256], f32, name="stgq", tag="stgq")
                nc.sync.dma_start(out=sm, in_=srow[0:1, 0:1024])
                nc.scalar.dma_start(out=sq, in_=srow[0:1, 1024:2048])
                u2 = stage.tile([4, 256], f32, name="u2", tag="u2")
                nc.vector.tensor_mul(u2, sm, sm)
                z = stage.tile([4, 256], f32, name="z", tag="z")
                nc.vector.scalar_tensor_tensor(
                    out=z, in0=u2, scalar=-1.0 / C, in1=sq,
                    op0=ALU.mult, op1=ALU.add,
                )
                nc.vector.tensor_scalar_add(z, z, C * EPS)
                r0i = stage.tile([4, 256], i32, name="r0i", tag="r0i")
                nc.vector.tensor_scalar(
                    out=r0i, in0=z.bitcast(i32), scalar1=1, scalar2=None,
                    op0=ALU.logical_shift_right,
                )
                nc.vector.tensor_sub(r0i, magict, r0i)
                r0 = r0i.bitcast(f32)
                a2 = stage.tile([4, 256], f32, name="a2", tag="a2")
                nc.vector.tensor_mul(a2, r0, r0)
                nc.vector.tensor_mul(a2, a2, z)
                nc.vector.tensor_scalar(
                    out=a2, in0=a2, scalar1=-0.5 * SQC, scalar2=1.5 * SQC,
                    op0=ALU.mult, op1=ALU.add,
                )
                rt = stage.tile([4, 256], f32, name="rt", tag="rt")
                nc.vector.tensor_mul(rt, r0, a2)
                # rt = rsqrt(var+eps) for positions qb*1024..+1024
                nc.scalar.dma_start(
                    out=r_dram[0:1, qb * 1024:(qb + 1) * 1024], in_=rt
                )
                # per-partition key layout: rc[p, c8] = r[c8*128 + p]
                rg = r_dram[0:1, qb * 1024:(qb + 1) * 1024].rearrange(
                    "o (c p) -> p (o c)", p=128
                )
                rcw = stage.tile([128, 8], f32, name="rcw", tag="rcw")
                nc.sync.dma_start(out=rcw, in_=rg)
                nc.vector.tensor_scalar_mul(
                    rc_v[:, qb * 8:(qb + 1) * 8], rcw, 1.0 / SQC
                )
                nc.vector.tensor_scalar_mul(
                    rc_q[:, qb * 8:(qb + 1) * 8], rcw, 1.0 / Q8S
                )
                if qb == 0:
                    nc.gpsimd.dma_start(
                        out=rr,
                        in_=r_dram[0:1, 0:NQ].to_broadcast([128, NQ]),
                    )

            def k_chunk(j):
                """K for 512-col chunk j: k8 = fp8(kp/sqrt(C)) (no r)."""
                for co in range(PC):
                    kp = psB.tile([128, CH], f32, tag="b", name="kp")
                    for i2 in range(2):
                        nc.tensor.matmul(
                            kp,
                            wk_sb[:, 2 * i2:2 * i2 + 2, co * 128:(co + 1) * 128],
                            x8[:, 2 * i2:2 * i2 + 2, j * CH:(j + 1) * CH],
                            start=(i2 == 0), stop=(i2 == 1), perf_mode=DR,
                        )
                    nc.scalar.activation(
                        k_all[:, co, j * CH:(j + 1) * CH], kp,
                        AF.Copy, scale=1.0 / SQC,
                    )

            def v_chunk(j):
                """V for chunk j: v8 = fp8(r_n/sqrt(C) * vp), [n, c] layout."""
                for s4 in range(4):
                    jk = 4 * j + s4
                    vp = psB.tile([128, C], f32, tag="b", name="vp")
                    for i2 in range(2):
                        nc.tensor.matmul(
                            vp,
                            x8[:, 2 * i2:2 * i2 + 2, jk * 128:(jk + 1) * 128],
                            wv_sb[:, 2 * i2:2 * i2 + 2, :],
                            start=(i2 == 0), stop=(i2 == 1), perf_mode=DR,
                        )
                    if s4 % 2 == 0:
                        nc.scalar.activation(
                            v_all[:, jk, :], vp, AF.Copy,
                            scale=rc_v[:, jk:jk + 1],
                        )
                    else:
                        nc.vector.tensor_scalar(
                            out=v_all[:, jk, :], in0=vp,
                            scalar1=rc_v[:, jk:jk + 1], scalar2=None,
                            op0=ALU.mult,
                        )

            def q_chunk(j):
                """Q for own 512-col chunk j (j in {0,1}), with r and bias."""
                for co in range(PC):
                    qp = psB.tile([128, CH], f32, tag="b", name="qp")
                    for i2 in range(2):
                        nc.tensor.matmul(
                            qp,
                            wq_sb[:, 2 * i2:2 * i2 + 2, co * 128:(co + 1) * 128],
                            x8[:, 2 * i2:2 * i2 + 2, j * CH:(j + 1) * CH],
                            start=(i2 == 0), stop=(i2 == 1), perf_mode=DR,
                        )
                    qt = x2p.tile([128, CH], f32, tag="qt", name="qt")
                    nc.vector.scalar_tensor_tensor(
                        out=qt, in0=qp, scalar=Q8S / C,
                        in1=rr[:, j * CH:(j + 1) * CH],
                        op0=ALU.mult, op1=ALU.mult,
                    )
                    nc.vector.tensor_scalar(
                        out=q_all[:, co, j * CH:(j + 1) * CH], in0=qt,
                        scalar1=1.0, scalar2=bq_sb[:, co:co + 1],
                        op0=ALU.mult, op1=ALU.add,
                    )

            # ---- attention ----
            def attention_group(g):
                q0 = g * 512
                avps = [
                    psB.tile([128, 512], f32, tag="b", name=f"avp{g}{s}")
                    for s in range(PC)
                ]
                sp = psA.tile([128, 512], f32, tag="a", name=f"sp{g}")
                for pr in range(16):
                    pt2 = ptp.tile([128, 2, 512], f8, tag="pt", name="pt2")
                    for u in range(2):
                        jk = 2 * pr + u
                        st = psC.tile([128, 512], f32, tag="c", name="st")
                        for i2 in range(2):
                            nc.tensor.matmul(
                                st,
                                k_all[:, 2 * i2:2 * i2 + 2,
                                      jk * 128:(jk + 1) * 128],
                                q_all[:, 2 * i2:2 * i2 + 2, q0:q0 + 512],
                                start=(i2 == 0), stop=(i2 == 1), perf_mode=DR,
                            )
                        nc.scalar.activation(
                            pt2[:, u, :], st, AF.Exp,
                            scale=rc_q[:, jk:jk + 1], bias=expb,
                        )
                    for ci in range(PC):
                        nc.tensor.matmul(
                            avps[ci],
                            v_all[:, 2 * pr:2 * pr + 2, ci * 128:(ci + 1) * 128],
                            pt2,
                            start=(pr == 0), stop=(pr == 15), perf_mode=DR,
                        )
                    nc.tensor.matmul(
                        sp, ones2, pt2,
                        start=(pr == 0), stop=(pr == 15), perf_mode=DR,
                    )
                # softmax normalize + project + residual
                # (every partition of sp carries the same sumexp row)
                rcb = small.tile([128, 512], f32, tag="rcb", name=f"rcb{g}")
                nc.vector.reciprocal_approx_fast(out=rcb, in_=sp)
                avn = avn_pool.tile([128, PC, 512], f8, name="avn")
                for ci in range(PC):
                    nc.vector.tensor_mul(avn[:, ci, :], avps[ci], rcb)
                out_t = out_pool.tile([128, PC, 512], f32, name="outt")
                for co in range(PC):
                    pop = psC.tile([128, 512], f32, tag="c", name="pop")
                    for i2 in range(2):
                        nc.tensor.matmul(
                            pop,
                            wp_sb[:, 2 * i2:2 * i2 + 2, co * 128:(co + 1) * 128],
                            avn[:, 2 * i2:2 * i2 + 2, :],
                            start=(i2 == 0), stop=(i2 == 1), perf_mode=DR,
                        )
                    nc.vector.tensor_scalar(
                        out=out_t[:, co, :], in0=pop,
                        scalar1=1.0 / SQC, scalar2=bp_sb[:, co:co + 1],
                        op0=ALU.mult, op1=ALU.add,
                    )
                    for h in range(2):
                        eng = nc.gpsimd if h == 0 else nc.vector
                        eng.tensor_add(
                            out_t[:, co, h * 256:(h + 1) * 256],
                            out_t[:, co, h * 256:(h + 1) * 256],
                            x_sb[:, co, q0 + h * 256:q0 + (h + 1) * 256],
                        )
                        (nc.sync if h == 0 else nc.scalar).dma_start(
                            out=outr[:, co, q0 + h * 256:q0 + (h + 1) * 256],
                            in_=out_t[:, co, h * 256:(h + 1) * 256],
                        )

            # ---- emission ----
            nc.scalar.dma_start(out=bq_sb, in_=bqd[:])
            nc.sync.dma_start(out=bp_sb, in_=bpd[:])
            nc.sync.dma_start(out=wk_sb, in_=wkd[:])
            nc.scalar.dma_start(out=wv_sb, in_=wvd[:])
            nc.scalar.dma_start(out=wq_sb, in_=wqd[:])
            nc.sync.dma_start(out=wp_sb, in_=wpd[:])
            for j in range(3):
                dmaeng[j % 3].dma_start(
                    out=x8[:, :, j * CH:(j + 1) * CH], in_=x8d[j]
                )

            stg = {}
            for j in range(NJ):
                if j + 3 < NJ:
                    dmaeng[(j + 3) % 3].dma_start(
                        out=x8[:, :, (j + 3) * CH:(j + 4) * CH],
                        in_=x8d[j + 3],
                    )
                # residual x chunks trickle in; only needed at the end
                dmaeng[j % 3].dma_start(
                    out=x_sb[:, :, j * CH:(j + 1) * CH], in_=xd[j]
                )
                x8q = x2p.tile([128, PC, CH], f8, name="x8q", tag="x8q")
                nc.gpsimd.dma_start(out=x8q, in_=xqd[j])
                qb = j // 2
                if j % 2 == 0:
                    stg[qb] = stage.tile([1, 2048], f32, name="sr", tag="sr")
                stats_chunk(j, stg[qb], x8q)
                k_chunk(j)
                if j % 2 == 1:
                    rchain(qb, stg[qb])
                if j >= 2:
                    v_chunk(j - 2)
                if j - 2 in (0, 1):
                    q_chunk(j - 2)
            v_chunk(6)
            v_chunk(7)
            attention_group(0)
            attention_group(1)

    nc.compile()
    return nc


def _get_nc():
    global _cached_nc
    if _cached_nc is None:
        _cached_nc = _build_nc()
    return _cached_nc


def kernel(x, norm_w, w_qkv, b_qkv, w_proj, b_proj):
    import ml_dtypes

    f8np = ml_dtypes.float8_e4m3

    x = np.asarray(x, dtype=np.float32)
    norm_w = np.asarray(norm_w, dtype=np.float32)
    w_qkv = np.asarray(w_qkv, dtype=np.float32)
    b_qkv = np.asarray(b_qkv, dtype=np.float32)
    w_proj = np.asarray(w_proj, dtype=np.float32)
    b_proj = np.asarray(b_proj, dtype=np.float32)

    B = x.shape[0]

    # fold norm_w + LN centering into the QKV weights; sqrt(C) into all
    Wq = w_qkv[0:C] * norm_w[None, :]
    Wk = w_qkv[C:2 * C] * norm_w[None, :]
    Wv = w_qkv[2 * C:3 * C] * norm_w[None, :]

    def wtile(wt):  # [cin, cout] -> [128, PC, cout]
        return np.ascontiguousarray(
            wt.reshape(PC, 128, C).transpose(1, 0, 2).astype(f8np)
        )

    Wqt = wtile(((Wq - Wq.mean(1, keepdims=True)) * SQC).T)
    Wkt = wtile(((Wk - Wk.mean(1, keepdims=True)) * SQC).T)
    Wvt = wtile(((Wv - Wv.mean(1, keepdims=True)) * SQC).T)
    Wpt = wtile(w_proj.T * SQC)

    def cols(b):  # [C] -> [128, 4] chunk-column layout
        return np.ascontiguousarray(b.reshape(PC, 128).T)

    bq = cols(b_qkv[0:C] * (Q8S / SQC))
    bv = b_qkv[2 * C:3 * C]
    bpt = cols(b_proj + w_proj @ bv)

    in_maps = []
    for core in range(8):
        bi, qi = core // 4, core % 4
        xl = np.roll(x[bi].reshape(C, N), -qi * NQ, axis=1)
        # pre-tile to the on-chip layout: [chunk, partition, c-chunk, col]
        xl = np.ascontiguousarray(
            xl.reshape(PC, 128, NJ, CH).transpose(2, 1, 0, 3)
        )
        x8l = xl.astype(f8np)
        x8sq = (x8l.astype(np.float32) ** 2).astype(f8np)
        in_maps.append({
            "x": xl, "x8": x8l, "xq8": x8sq,
            "wq": Wqt, "wk": Wkt, "wv": Wvt, "wp": Wpt,
            "bq": bq, "bp": bpt,
        })

    from concourse.bass_utils import run_bass_kernel_spmd

    nc = _get_nc()
    res = run_bass_kernel_spmd(nc, in_maps, core_ids=list(range(8)))

    out = np.empty((B, C, N), dtype=np.float32)
    for core in range(8):
        bi, qi = core // 4, core % 4
        out[bi][:, qi * NQ:(qi + 1) * NQ] = res.results[core]["out"]
    return out.reshape(x.shape)


# revision 11
# speedup vs baseline: 1.8257x; 1.1822x over previous
"""AttentionBlock (b=2, c=512, 64x64) on 8 trn2 NeuronCores.

Sharding: core i handles batch i//4, query rows (i%4)*1024..+1024 (of the
4096 flattened h*w positions). Each core receives its batch's full x with
columns rotated so its own query block sits at columns 0:1024, computes
LayerNorm stats + K + V for all 4096 positions (replicated inside the
4-core batch group) and Q/attention/projection for its 1024 queries.

Key structural points (v2):
  - x8 = fp8(x) straight away; the QKV matmuls do NOT wait on the
    LayerNorm stats.  The rsqrt(var+eps) factor r is applied after the
    matmuls instead:
      * K is evicted UNscaled (k8 = fp8(kp/sqrt(C))); the per-key factor
        r_k is folded into the softmax exp as a per-partition ACT scale
        (st has keys on partitions).
      * V is scaled at eviction by a per-partition scalar r_n.
      * Q is scaled by a broadcast row of r (its own 1024 positions only)
        plus the folded bias.
  - k-bias drops out of softmax entirely; v-bias is folded into the
    projection bias on the host; q-bias is kept (folded scale).
  - Attention AV matmuls run with V stationary, so the attention output
    lands directly in [c, q] layout: no PE transposes, and the
    projection consumes it as the moving operand straight away.
  - sumexp accumulates in PSUM across all 16 key-pair steps.
  - PSUM budget (8 banks): psA(2) stats ping/pong then sumexp;
    psB(4) kp/vp/qp rotation then the 4 avps accumulators;
    psC(2) st ping/pong then proj outputs.
"""
import sys

if "/opt/trn_rl_repo" not in sys.path:
    sys.path.insert(0, "/opt/trn_rl_repo")

import numpy as np

C = 512          # channels
N = 4096         # h*w positions
NQ = 1024        # queries per core
PC = 4           # c chunks of 128
NKC = 32         # key chunks of 128
NJ = 8           # x column chunks of 512
CH = 512         # x chunk width
EPS = 1e-5
SQC = 22.627416997969522   # sqrt(512)
Q8S = 32.0                 # q8 = 32/sqrt(C) * logit-ready q
EXPB = -1.5                # exp(logits + EXPB), cancels in softmax
MAGIC = 0x5F3759DF         # Quake rsqrt seed

_cached_nc = None


def _build_nc():
    import concourse.bass as bass
    import concourse.tile as tile
    from concourse import bacc, mybir

    f32 = mybir.dt.float32
    f32r = mybir.dt.float32r
    i32 = mybir.dt.int32
    f8 = mybir.dt.float8e4
    AF = mybir.ActivationFunctionType
    ALU = mybir.AluOpType
    DR = mybir.MatmulPerfMode.DoubleRow

    nc = bacc.Bacc(None, target_bir_lowering=False)

    xd = nc.declare_dram_parameter("x", [NJ, 128, PC, CH], f32, isOutput=False)
    x8d = nc.declare_dram_parameter("x8", [NJ, 128, PC, CH], f8, isOutput=False)
    xqd = nc.declare_dram_parameter("xq8", [NJ, 128, PC, CH], f8, isOutput=False)
    wqd = nc.declare_dram_parameter("wq", [128, PC, C], f8, isOutput=False)
    wkd = nc.declare_dram_parameter("wk", [128, PC, C], f8, isOutput=False)
    wvd = nc.declare_dram_parameter("wv", [128, PC, C], f8, isOutput=False)
    wpd = nc.declare_dram_parameter("wp", [128, PC, C], f8, isOutput=False)
    bqd = nc.declare_dram_parameter("bq", [128, PC], f32, isOutput=False)
    bpd = nc.declare_dram_parameter("bp", [128, PC], f32, isOutput=False)
    outd = nc.declare_dram_parameter("out", [C, NQ], f32, isOutput=True)

    outr = outd.rearrange("(a p) n -> p a n", p=128)   # [128, 4, NQ]

    with tile.TileContext(nc) as tc:
        from contextlib import ExitStack

        with ExitStack() as ctx:
            consts = ctx.enter_context(tc.tile_pool(name="consts", bufs=1))
            xpool = ctx.enter_context(tc.tile_pool(name="xpool", bufs=1))
            kvq = ctx.enter_context(tc.tile_pool(name="kvq", bufs=1))
            dramp = ctx.enter_context(
                tc.tile_pool(name="dramp", bufs=1, space="DRAM")
            )
            # PSUM: exactly 8 banks
            psA = ctx.enter_context(
                tc.tile_pool(name="psA", bufs=2, space=bass.MemorySpace.PSUM)
            )
            psB = ctx.enter_context(
                tc.tile_pool(name="psB", bufs=4, space=bass.MemorySpace.PSUM)
            )
            psC = ctx.enter_context(
                tc.tile_pool(name="psC", bufs=2, space=bass.MemorySpace.PSUM)
            )
            stage = ctx.enter_context(tc.tile_pool(name="stage", bufs=2))
            x2p = ctx.enter_context(tc.tile_pool(name="x2p", bufs=2))
            ptp = ctx.enter_context(tc.tile_pool(name="ptp", bufs=4))
            avn_pool = ctx.enter_context(tc.tile_pool(name="avn", bufs=2))
            out_pool = ctx.enter_context(tc.tile_pool(name="outp", bufs=1))
            small = ctx.enter_context(tc.tile_pool(name="small", bufs=2))

            ones2 = consts.tile([128, 2, 128], f8)
            nc.vector.memset(ones2, 1.0)
            magict = consts.tile([4, 256], i32)
            nc.vector.memset(magict, MAGIC)
            expb = consts.tile([128, 1], f32)
            nc.vector.memset(expb, EXPB)

            bq_sb = consts.tile([128, PC], f32)
            bp_sb = consts.tile([128, PC], f32)
            wq_sb = consts.tile([128, PC, C], f8)
            wk_sb = consts.tile([128, PC, C], f8)
            wv_sb = consts.tile([128, PC, C], f8)
            wp_sb = consts.tile([128, PC, C], f8)

            x_sb = xpool.tile([128, PC, N], f32)
            x8 = kvq.tile([128, PC, N], f8)
            k_all = kvq.tile([128, PC, N], f8)     # (c, n) layout
            v_all = kvq.tile([128, NKC, C], f8)    # (n, c) layout
            q_all = kvq.tile([128, PC, NQ], f8)    # (c, nq) layout
            rc_v = kvq.tile([128, NKC], f32)       # r/sqrt(C), keyed [p, chunk]
            rc_q = kvq.tile([128, NKC], f32)       # r/Q8S, keyed [p, chunk]
            rr = kvq.tile([128, NQ], f32)          # r broadcast, own queries

            r_dram = dramp.tile([1, N], f32)

            dmaeng = [nc.sync, nc.scalar, nc.gpsimd]

            # ---- phase 1 helpers ----
            def stats_chunk(j, srow, x8q):
                """Column sums of x8 and x8^2 for 512-col chunk j (fp8 DR)."""
                ps_s = psA.tile([128, CH], f32, tag="a", name="ps_s")
                ps_q = psA.tile([128, CH], f32, tag="a", name="ps_q")
                for i2 in range(2):
                    nc.tensor.matmul(
                        ps_s, ones2,
                        x8[:, 2 * i2:2 * i2 + 2, j * CH:(j + 1) * CH],
                        start=(i2 == 0), stop=(i2 == 1), perf_mode=DR,
                    )
                for i2 in range(2):
                    nc.tensor.matmul(
                        ps_q, ones2, x8q[:, 2 * i2:2 * i2 + 2, :],
                        start=(i2 == 0), stop=(i2 == 1), perf_mode=DR,
                    )
                h = CH * (j % 2)
                nc.vector.tensor_copy(srow[0:1, h:h + CH], ps_s[0:1, :])
                nc.vector.tensor_copy(
                    srow[0:1, 1024 + h:1024 + h + CH], ps_q[0:1, :]
                )

            def rchain(qb, srow):
                """r = rsqrt(var+eps) for quarter qb -> r_dram + rc/rr tiles.
                Quake rsqrt + 1 Newton step, DVE only."""
                sm = stage.tile([4, 256], f32, name="stgs", tag="stgs")
                sq = stage.tile([4, 256], f32, name="stgq", tag="stgq")
                nc.sync.dma_start(out=sm, in_=srow[0:1, 0:1024])
                nc.sync.dma_start(out=sq, in_=srow[0:1, 1024:2048])
                u2 = stage.tile([4, 256], f32, name="u2", tag="u2")
                nc.vector.tensor_mul(u2, sm, sm)
                z = stage.tile([4, 256], f32, name="z", tag="z")
                nc.vector.scalar_tensor_tensor(
                    out=z, in0=u2, scalar=-1.0 / C, in1=sq,
                    op0=ALU.mult, op1=ALU.add,
                )
                nc.vector.tensor_scalar_add(z, z, C * EPS)
                r0i = stage.tile([4, 256], i32, name="r0i", tag="r0i")
                nc.vector.tensor_scalar(
                    out=r0i, in0=z.bitcast(i32), scalar1=1, scalar2=None,
                    op0=ALU.logical_shift_right,
                )
                nc.vector.tensor_sub(r0i, magict, r0i)
                r0 = r0i.bitcast(f32)
                a2 = stage.tile([4, 256], f32, name="a2", tag="a2")
                nc.vector.tensor_mul(a2, r0, r0)
                nc.vector.tensor_mul(a2, a2, z)
                nc.vector.tensor_scalar(
                    out=a2, in0=a2, scalar1=-0.5 * SQC, scalar2=1.5 * SQC,
                    op0=ALU.mult, op1=ALU.add,
                )
                rt = stage.tile([4, 256], f32, name="rt", tag="rt")
                nc.vector.tensor_mul(rt, r0, a2)
                # rt = rsqrt(var+eps) for positions qb*1024..+1024
                nc.gpsimd.dma_start(
                    out=r_dram[0:1, qb * 1024:(qb + 1) * 1024], in_=rt
                )
                # per-partition key layout: rc[p, c8] = r[c8*128 + p]
                rg = r_dram[0:1, qb * 1024:(qb + 1) * 1024].rearrange(
                    "o (c p) -> p (o c)", p=128
                )
                rcw = stage.tile([128, 8], f32, name="rcw", tag="rcw")
                nc.gpsimd.dma_start(out=rcw, in_=rg)
                nc.vector.tensor_scalar_mul(
                    rc_v[:, qb * 8:(qb + 1) * 8], rcw, 1.0 / SQC
                )
                nc.vector.tensor_scalar_mul(
                    rc_q[:, qb * 8:(qb + 1) * 8], rcw, 1.0 / Q8S
                )
                if qb == 0:
                    nc.gpsimd.dma_start(
                        out=rr,
                        in_=r_dram[0:1, 0:NQ].to_broadcast([128, NQ]),
                    )

            def k_chunk(j):
                """K for 512-col chunk j: k8 = fp8(kp/sqrt(C)) (no r)."""
                for co in range(PC):
                    kp = psB.tile([128, CH], f32, tag="b", name="kp")
                    for i2 in range(2):
                        nc.tensor.matmul(
                            kp,
                            wk_sb[:, 2 * i2:2 * i2 + 2, co * 128:(co + 1) * 128],
                            x8[:, 2 * i2:2 * i2 + 2, j * CH:(j + 1) * CH],
                            start=(i2 == 0), stop=(i2 == 1), perf_mode=DR,
                        )
                    nc.scalar.activation(
                        k_all[:, co, j * CH:(j + 1) * CH], kp,
                        AF.Copy, scale=1.0 / SQC,
                    )

            def v_chunk(j):
                """V for chunk j: v8 = fp8(r_n/sqrt(C) * vp), [n, c] layout."""
                for s4 in range(4):
                    jk = 4 * j + s4
                    vp = psB.tile([128, C], f32, tag="b", name="vp")
                    for i2 in range(2):
                        nc.tensor.matmul(
                            vp,
                            x8[:, 2 * i2:2 * i2 + 2, jk * 128:(jk + 1) * 128],
                            wv_sb[:, 2 * i2:2 * i2 + 2, :],
                            start=(i2 == 0), stop=(i2 == 1), perf_mode=DR,
                        )
                    if s4 % 2 == 0:
                        nc.scalar.activation(
                            v_all[:, jk, :], vp, AF.Copy,
                            scale=rc_v[:, jk:jk + 1],
                        )
                    else:
                        nc.vector.tensor_scalar(
                            out=v_all[:, jk, :], in0=vp,
                            scalar1=rc_v[:, jk:jk + 1], scalar2=None,
                            op0=ALU.mult,
                        )

            def q_chunk(j):
                """Q for own 512-col chunk j (j in {0,1}), with r and bias."""
                for co in range(PC):
                    qp = psB.tile([128, CH], f32, tag="b", name="qp")
                    for i2 in range(2):
                        nc.tensor.matmul(
                            qp,
                            wq_sb[:, 2 * i2:2 * i2 + 2, co * 128:(co + 1) * 128],
                            x8[:, 2 * i2:2 * i2 + 2, j * CH:(j + 1) * CH],
                            start=(i2 == 0), stop=(i2 == 1), perf_mode=DR,
                        )
                    qt = x2p.tile([128, CH], f32, tag="qt", name="qt")
                    nc.vector.scalar_tensor_tensor(
                        out=qt, in0=qp, scalar=Q8S / C,
                        in1=rr[:, j * CH:(j + 1) * CH],
                        op0=ALU.mult, op1=ALU.mult,
                    )
                    nc.vector.tensor_scalar(
                        out=q_all[:, co, j * CH:(j + 1) * CH], in0=qt,
                        scalar1=1.0, scalar2=bq_sb[:, co:co + 1],
                        op0=ALU.mult, op1=ALU.add,
                    )

            # ---- attention ----
            def attention_group(g):
                q0 = g * 512
                avps = [
                    psB.tile([128, 512], f32, tag="b", name=f"avp{g}{s}")
                    for s in range(PC)
                ]
                sp = psA.tile([128, 512], f32, tag="a", name=f"sp{g}")
                for pr in range(16):
                    pt2 = ptp.tile([128, 2, 512], f8, tag="pt", name="pt2")
                    for u in range(2):
                        jk = 2 * pr + u
                        stp = psA if (2 * pr + u) % 3 == 2 else psC
                        st = stp.tile(
                            [128, 512], f32,
                            tag="a" if stp is psA else "c", name="st",
                        )
                        for i2 in range(2):
                            nc.tensor.matmul(
                                st,
                                k_all[:, 2 * i2:2 * i2 + 2,
                                      jk * 128:(jk + 1) * 128],
                                q_all[:, 2 * i2:2 * i2 + 2, q0:q0 + 512],
                                start=(i2 == 0), stop=(i2 == 1), perf_mode=DR,
                            )
                        nc.scalar.activation(
                            pt2[:, u, :], st, AF.Exp,
                            scale=rc_q[:, jk:jk + 1], bias=expb,
                        )
                    for ci in range(PC):
                        nc.tensor.matmul(
                            avps[ci],
                            v_all[:, 2 * pr:2 * pr + 2, ci * 128:(ci + 1) * 128],
                            pt2,
                            start=(pr == 0), stop=(pr == 15), perf_mode=DR,
                        )
                    nc.tensor.matmul(
                        sp, ones2, pt2,
                        start=(pr == 0), stop=(pr == 15), perf_mode=DR,
                    )
                # softmax normalize + project + residual
                # (every partition of sp carries the same sumexp row)
                rcb = small.tile([128, 512], f32, tag="rcb", name=f"rcb{g}")
                nc.vector.reciprocal_approx_fast(out=rcb, in_=sp)
                avn = avn_pool.tile([128, PC, 512], f8, name="avn")
                for ci in range(PC):
                    nc.vector.tensor_mul(avn[:, ci, :], avps[ci], rcb)
                out_t = out_pool.tile([128, PC, 512], f32, name="outt")
                for co in range(PC):
                    pop = psC.tile([128, 512], f32, tag="c", name="pop")
                    for i2 in range(2):
                        nc.tensor.matmul(
                            pop,
                            wp_sb[:, 2 * i2:2 * i2 + 2, co * 128:(co + 1) * 128],
                            avn[:, 2 * i2:2 * i2 + 2, :],
                            start=(i2 == 0), stop=(i2 == 1), perf_mode=DR,
                        )
                    nc.vector.tensor_scalar(
                        out=out_t[:, co, :], in0=pop,
                        scalar1=1.0 / SQC, scalar2=bp_sb[:, co:co + 1],
                        op0=ALU.mult, op1=ALU.add,
                    )
                    for h in range(2):
                        eng = nc.gpsimd if h == 0 else nc.vector
                        eng.tensor_add(
                            out_t[:, co, h * 256:(h + 1) * 256],
                            out_t[:, co, h * 256:(h + 1) * 256],
                            x_sb[:, co, q0 + h * 256:q0 + (h + 1) * 256],
                        )
                        (nc.sync if h == 0 else nc.scalar).dma_start(
                            out=outr[:, co, q0 + h * 256:q0 + (h + 1) * 256],
                            in_=out_t[:, co, h * 256:(h + 1) * 256],
                        )

            # ---- emission ----
            nc.sync.dma_start(out=x8[:, :, 0:CH], in_=x8d[0])
            nc.scalar.dma_start(out=wk_sb, in_=wkd[:])
            nc.gpsimd.dma_start(out=wv_sb, in_=wvd[:])
            nc.sync.dma_start(out=x8[:, :, CH:2 * CH], in_=x8d[1])
            nc.scalar.dma_start(out=wq_sb, in_=wqd[:])
            nc.scalar.dma_start(out=bq_sb, in_=bqd[:])
            nc.sync.dma_start(out=x8[:, :, 2 * CH:3 * CH], in_=x8d[2])

            stg = {}
            for j in range(NJ):
                if j + 3 < NJ:
                    nc.sync.dma_start(
                        out=x8[:, :, (j + 3) * CH:(j + 4) * CH],
                        in_=x8d[j + 3],
                    )
                # residual x chunks trickle in; only needed at the end
                x8q = x2p.tile([128, PC, CH], f8, name="x8q", tag="x8q")
                nc.gpsimd.dma_start(out=x8q, in_=xqd[j])
                nc.gpsimd.dma_start(
                    out=x_sb[:, :, j * CH:(j + 1) * CH], in_=xd[j]
                )
                qb = j // 2
                if j % 2 == 0:
                    stg[qb] = stage.tile([1, 2048], f32, name="sr", tag="sr")
                stats_chunk(j, stg[qb], x8q)
                k_chunk(j)
                if j % 2 == 1:
                    rchain(qb, stg[qb])
                if j >= 3:
                    v_chunk(j - 3)
                if j - 3 in (0, 1):
                    q_chunk(j - 3)
                if j == NJ - 1:
                    nc.sync.dma_start(out=wp_sb, in_=wpd[:])
                    nc.scalar.dma_start(out=bp_sb, in_=bpd[:])
            for j in range(5, 8):
                v_chunk(j)
            attention_group(0)
            attention_group(1)

    nc.compile()
    return nc


def _get_nc():
    global _cached_nc
    if _cached_nc is None:
        _cached_nc = _build_nc()
    return _cached_nc


def kernel(x, norm_w, w_qkv, b_qkv, w_proj, b_proj):
    import ml_dtypes

    f8np = ml_dtypes.float8_e4m3

    x = np.asarray(x, dtype=np.float32)
    norm_w = np.asarray(norm_w, dtype=np.float32)
    w_qkv = np.asarray(w_qkv, dtype=np.float32)
    b_qkv = np.asarray(b_qkv, dtype=np.float32)
    w_proj = np.asarray(w_proj, dtype=np.float32)
    b_proj = np.asarray(b_proj, dtype=np.float32)

    B = x.shape[0]

    # fold norm_w + LN centering into the QKV weights; sqrt(C) into all
    Wq = w_qkv[0:C] * norm_w[None, :]
    Wk = w_qkv[C:2 * C] * norm_w[None, :]
    Wv = w_qkv[2 * C:3 * C] * norm_w[None, :]

    def wtile(wt):  # [cin, cout] -> [128, PC, cout]
        return np.ascontiguousarray(
            wt.reshape(PC, 128, C).transpose(1, 0, 2).astype(f8np)
        )

    Wqt = wtile(((Wq - Wq.mean(1, keepdims=True)) * SQC).T)
    Wkt = wtile(((Wk - Wk.mean(1, keepdims=True)) * SQC).T)
    Wvt = wtile(((Wv - Wv.mean(1, keepdims=True)) * SQC).T)
    Wpt = wtile(w_proj.T * SQC)

    def cols(b):  # [C] -> [128, 4] chunk-column layout
        return np.ascontiguousarray(b.reshape(PC, 128).T)

    bq = cols(b_qkv[0:C] * (Q8S / SQC))
    bv = b_qkv[2 * C:3 * C]
    bpt = cols(b_proj + w_proj @ bv)

    in_maps = []
    for core in range(8):
        bi, qi = core // 4, core % 4
        xl = np.roll(x[bi].reshape(C, N), -qi * NQ, axis=1)
        # pre-tile to the on-chip layout: [chunk, partition, c-chunk, col]
        xl = np.ascontiguousarray(
            xl.reshape(PC, 128, NJ, CH).transpose(2, 1, 0, 3)
        )
        x8l = xl.astype(f8np)
        x8sq = (x8l.astype(np.float32) ** 2).astype(f8np)
        in_maps.append({
            "x": xl, "x8": x8l, "xq8": x8sq,
            "wq": Wqt, "wk": Wkt, "wv": Wvt, "wp": Wpt,
            "bq": bq, "bp": bpt,
        })

    from concourse.bass_utils import run_bass_kernel_spmd

    nc = _get_nc()
    res = run_bass_kernel_spmd(nc, in_maps, core_ids=list(range(8)))

    out = np.empty((B, C, N), dtype=np.float32)
    for core in range(8):
        bi, qi = core // 4, core % 4
        out[bi][:, qi * NQ:(qi + 1) * NQ] = res.results[core]["out"]
    return out.reshape(x.shape)


# revision 12
# speedup vs baseline: 1.9474x; 1.0666x over previous
"""AttentionBlock (b=2, c=512, 64x64) on 8 trn2 NeuronCores.

Sharding: core i handles batch i//4, query rows (i%4)*1024..+1024 (of the
4096 flattened h*w positions). Each core receives its batch's full x with
columns rotated so its own query block sits at columns 0:1024, computes
LayerNorm stats + K + V for all 4096 positions (replicated inside the
4-core batch group) and Q/attention/projection for its 1024 queries.

Key structural points (v2):
  - x8 = fp8(x) straight away; the QKV matmuls do NOT wait on the
    LayerNorm stats.  The rsqrt(var+eps) factor r is applied after the
    matmuls instead:
      * K is evicted UNscaled (k8 = fp8(kp/sqrt(C))); the per-key factor
        r_k is folded into the softmax exp as a per-partition ACT scale
        (st has keys on partitions).
      * V is scaled at eviction by a per-partition scalar r_n.
      * Q is scaled by a broadcast row of r (its own 1024 positions only)
        plus the folded bias.
  - k-bias drops out of softmax entirely; v-bias is folded into the
    projection bias on the host; q-bias is kept (folded scale).
  - Attention AV matmuls run with V stationary, so the attention output
    lands directly in [c, q] layout: no PE transposes, and the
    projection consumes it as the moving operand straight away.
  - sumexp accumulates in PSUM across all 16 key-pair steps.
  - PSUM budget (8 banks): psA(2) stats ping/pong then sumexp;
    psB(4) kp/vp/qp rotation then the 4 avps accumulators;
    psC(2) st ping/pong then proj outputs.
"""
import sys

if "/opt/trn_rl_repo" not in sys.path:
    sys.path.insert(0, "/opt/trn_rl_repo")

import numpy as np

C = 512          # channels
N = 4096         # h*w positions
NQ = 1024        # queries per core
PC = 4           # c chunks of 128
NKC = 32         # key chunks of 128
NJ = 8           # x column chunks of 512
CH = 512         # x chunk width
EPS = 1e-5
SQC = 22.627416997969522   # sqrt(512)
Q8S = 32.0                 # q8 = 32/sqrt(C) * logit-ready q
EXPB = -1.5                # exp(logits + EXPB), cancels in softmax
MAGIC = 0x5F3759DF         # Quake rsqrt seed

_cached_nc = None


def _build_nc():
    import concourse.bass as bass
    import concourse.tile as tile
    from concourse import bacc, mybir

    f32 = mybir.dt.float32
    f32r = mybir.dt.float32r
    i32 = mybir.dt.int32
    f8 = mybir.dt.float8e4
    AF = mybir.ActivationFunctionType
    ALU = mybir.AluOpType
    DR = mybir.MatmulPerfMode.DoubleRow

    nc = bacc.Bacc(None, target_bir_lowering=False)

    xd = nc.declare_dram_parameter("x", [NJ, 128, PC, CH], f32, isOutput=False)
    x8d = nc.declare_dram_parameter("x8", [NJ, 128, PC, CH], f8, isOutput=False)
    xqd = nc.declare_dram_parameter("xq8", [NJ, 128, PC, CH], f8, isOutput=False)
    wqd = nc.declare_dram_parameter("wq", [128, PC, C], f8, isOutput=False)
    wkd = nc.declare_dram_parameter("wk", [128, PC, C], f8, isOutput=False)
    wvd = nc.declare_dram_parameter("wv", [128, PC, C], f8, isOutput=False)
    wpd = nc.declare_dram_parameter("wp", [128, PC, C], f8, isOutput=False)
    bqd = nc.declare_dram_parameter("bq", [128, PC], f32, isOutput=False)
    bpd = nc.declare_dram_parameter("bp", [128, PC], f32, isOutput=False)
    outd = nc.declare_dram_parameter("out", [C, NQ], f32, isOutput=True)

    outr = outd.rearrange("(a p) n -> p a n", p=128)   # [128, 4, NQ]

    with tile.TileContext(nc) as tc:
        from contextlib import ExitStack

        with ExitStack() as ctx:
            consts = ctx.enter_context(tc.tile_pool(name="consts", bufs=1))
            xpool = ctx.enter_context(tc.tile_pool(name="xpool", bufs=1))
            kvq = ctx.enter_context(tc.tile_pool(name="kvq", bufs=1))
            dramp = ctx.enter_context(
                tc.tile_pool(name="dramp", bufs=1, space="DRAM")
            )
            # PSUM: exactly 8 banks
            psA = ctx.enter_context(
                tc.tile_pool(name="psA", bufs=2, space=bass.MemorySpace.PSUM)
            )
            psB = ctx.enter_context(
                tc.tile_pool(name="psB", bufs=4, space=bass.MemorySpace.PSUM)
            )
            psC = ctx.enter_context(
                tc.tile_pool(name="psC", bufs=2, space=bass.MemorySpace.PSUM)
            )
            stage = ctx.enter_context(tc.tile_pool(name="stage", bufs=2))
            x2p = ctx.enter_context(tc.tile_pool(name="x2p", bufs=2))
            ptp = ctx.enter_context(tc.tile_pool(name="ptp", bufs=4))
            avn_pool = ctx.enter_context(tc.tile_pool(name="avn", bufs=2))
            out_pool = ctx.enter_context(tc.tile_pool(name="outp", bufs=1))
            small = ctx.enter_context(tc.tile_pool(name="small", bufs=2))

            ones2 = consts.tile([128, 2, 128], f8)
            nc.vector.memset(ones2, 1.0)
            magict = consts.tile([4, 256], i32)
            nc.vector.memset(magict, MAGIC)
            expb = consts.tile([128, 1], f32)
            nc.vector.memset(expb, EXPB)

            bq_sb = consts.tile([128, PC], f32)
            bp_sb = consts.tile([128, PC], f32)
            wq_sb = consts.tile([128, PC, C], f8)
            wk_sb = consts.tile([128, PC, C], f8)
            wv_sb = consts.tile([128, PC, C], f8)
            wp_sb = consts.tile([128, PC, C], f8)

            x_sb = xpool.tile([128, PC, N], f32)
            x8 = kvq.tile([128, PC, N], f8)
            k_all = kvq.tile([128, PC, N], f8)     # (c, n) layout
            v_all = kvq.tile([128, NKC, C], f8)    # (n, c) layout
            q_all = kvq.tile([128, PC, NQ], f8)    # (c, nq) layout
            rc_v = kvq.tile([128, NKC], f32)       # r/sqrt(C), keyed [p, chunk]
            rc_q = kvq.tile([128, NKC], f32)       # r/Q8S, keyed [p, chunk]
            rr = kvq.tile([128, NQ], f32)          # r broadcast, own queries

            r_dram = dramp.tile([1, N], f32)

            dmaeng = [nc.sync, nc.scalar, nc.gpsimd]

            # ---- phase 1 helpers ----
            def stats_chunk(j, srow, sm, sq, x8q):
                """Column sums of x8 and x8^2 for 512-col chunk j (fp8 DR)."""
                ps_s = psA.tile([128, CH], f32, tag="a", name="ps_s")
                ps_q = psA.tile([128, CH], f32, tag="a", name="ps_q")
                for i2 in range(2):
                    nc.tensor.matmul(
                        ps_s, ones2,
                        x8[:, 2 * i2:2 * i2 + 2, j * CH:(j + 1) * CH],
                        start=(i2 == 0), stop=(i2 == 1), perf_mode=DR,
                    )
                for i2 in range(2):
                    nc.tensor.matmul(
                        ps_q, ones2, x8q[:, 2 * i2:2 * i2 + 2, :],
                        start=(i2 == 0), stop=(i2 == 1), perf_mode=DR,
                    )
                nc.scalar.activation(srow[0:1, 0:CH], ps_s[0:1, :], AF.Copy)
                nc.scalar.activation(
                    srow[0:1, CH:2 * CH], ps_q[0:1, :], AF.Copy
                )
                h = 2 * (j % 2)
                nc.sync.dma_start(out=sm[h:h + 2, :], in_=srow[0:1, 0:CH])
                nc.sync.dma_start(
                    out=sq[h:h + 2, :], in_=srow[0:1, CH:2 * CH]
                )

            def rchain(qb, sm, sq):
                """r = rsqrt(var+eps) for quarter qb -> r_dram + rc/rr tiles.
                Quake rsqrt + 1 Newton step, DVE only."""
                u2 = stage.tile([4, 256], f32, name="u2", tag="u2")
                nc.vector.tensor_mul(u2, sm, sm)
                z = stage.tile([4, 256], f32, name="z", tag="z")
                nc.vector.scalar_tensor_tensor(
                    out=z, in0=u2, scalar=-1.0 / C, in1=sq,
                    op0=ALU.mult, op1=ALU.add,
                )
                nc.vector.tensor_scalar_add(z, z, C * EPS)
                r0i = stage.tile([4, 256], i32, name="r0i", tag="r0i")
                nc.vector.tensor_scalar(
                    out=r0i, in0=z.bitcast(i32), scalar1=1, scalar2=None,
                    op0=ALU.logical_shift_right,
                )
                nc.vector.tensor_sub(r0i, magict, r0i)
                r0 = r0i.bitcast(f32)
                a2 = stage.tile([4, 256], f32, name="a2", tag="a2")
                nc.vector.tensor_mul(a2, r0, r0)
                nc.vector.tensor_mul(a2, a2, z)
                nc.vector.tensor_scalar(
                    out=a2, in0=a2, scalar1=-0.5 * SQC, scalar2=1.5 * SQC,
                    op0=ALU.mult, op1=ALU.add,
                )
                rt = stage.tile([4, 256], f32, name="rt", tag="rt")
                nc.vector.tensor_mul(rt, r0, a2)
                # rt = rsqrt(var+eps) for positions qb*1024..+1024
                nc.gpsimd.dma_start(
                    out=r_dram[0:1, qb * 1024:(qb + 1) * 1024], in_=rt
                )
                # per-partition key layout: rc[p, c8] = r[c8*128 + p]
                rg = r_dram[0:1, qb * 1024:(qb + 1) * 1024].rearrange(
                    "o (c p) -> p (o c)", p=128
                )
                rcw = stage.tile([128, 8], f32, name="rcw", tag="rcw")
                nc.gpsimd.dma_start(out=rcw, in_=rg)
                nc.vector.tensor_scalar_mul(
                    rc_v[:, qb * 8:(qb + 1) * 8], rcw, 1.0 / SQC
                )
                nc.vector.tensor_scalar_mul(
                    rc_q[:, qb * 8:(qb + 1) * 8], rcw, 1.0 / Q8S
                )
                if qb == 0:
                    nc.gpsimd.dma_start(
                        out=rr,
                        in_=r_dram[0:1, 0:NQ].to_broadcast([128, NQ]),
                    )

            def k_chunk(j):
                """K for 512-col chunk j: k8 = fp8(kp/sqrt(C)) (no r)."""
                for co in range(PC):
                    kp = psB.tile([128, CH], f32, tag="b", name="kp")
                    for i2 in range(2):
                        nc.tensor.matmul(
                            kp,
                            wk_sb[:, 2 * i2:2 * i2 + 2, co * 128:(co + 1) * 128],
                            x8[:, 2 * i2:2 * i2 + 2, j * CH:(j + 1) * CH],
                            start=(i2 == 0), stop=(i2 == 1), perf_mode=DR,
                        )
                    nc.scalar.activation(
                        k_all[:, co, j * CH:(j + 1) * CH], kp,
                        AF.Copy, scale=1.0 / SQC,
                    )

            def v_chunk(j):
                """V for chunk j: v8 = fp8(r_n/sqrt(C) * vp), [n, c] layout."""
                for s4 in range(4):
                    jk = 4 * j + s4
                    vp = psB.tile([128, C], f32, tag="b", name="vp")
                    for i2 in range(2):
                        nc.tensor.matmul(
                            vp,
                            x8[:, 2 * i2:2 * i2 + 2, jk * 128:(jk + 1) * 128],
                            wv_sb[:, 2 * i2:2 * i2 + 2, :],
                            start=(i2 == 0), stop=(i2 == 1), perf_mode=DR,
                        )
                    if s4 % 2 == 0:
                        nc.scalar.activation(
                            v_all[:, jk, :], vp, AF.Copy,
                            scale=rc_v[:, jk:jk + 1],
                        )
                    else:
                        nc.vector.tensor_scalar(
                            out=v_all[:, jk, :], in0=vp,
                            scalar1=rc_v[:, jk:jk + 1], scalar2=None,
                            op0=ALU.mult,
                        )

            def q_chunk(j):
                """Q for own 512-col chunk j (j in {0,1}), with r and bias."""
                for co in range(PC):
                    qp = psB.tile([128, CH], f32, tag="b", name="qp")
                    for i2 in range(2):
                        nc.tensor.matmul(
                            qp,
                            wq_sb[:, 2 * i2:2 * i2 + 2, co * 128:(co + 1) * 128],
                            x8[:, 2 * i2:2 * i2 + 2, j * CH:(j + 1) * CH],
                            start=(i2 == 0), stop=(i2 == 1), perf_mode=DR,
                        )
                    qt = x2p.tile([128, CH], f32, tag="qt", name="qt")
                    nc.vector.scalar_tensor_tensor(
                        out=qt, in0=qp, scalar=Q8S / C,
                        in1=rr[:, j * CH:(j + 1) * CH],
                        op0=ALU.mult, op1=ALU.mult,
                    )
                    nc.vector.tensor_scalar(
                        out=q_all[:, co, j * CH:(j + 1) * CH], in0=qt,
                        scalar1=1.0, scalar2=bq_sb[:, co:co + 1],
                        op0=ALU.mult, op1=ALU.add,
                    )

            # ---- attention ----
            def attention_group(g):
                q0 = g * 512
                avps = [
                    psB.tile([128, 512], f32, tag="b", name=f"avp{g}{s}")
                    for s in range(PC)
                ]
                sp = psA.tile([128, 512], f32, tag="a", name=f"sp{g}")
                for pr in range(16):
                    pt2 = ptp.tile([128, 2, 512], f8, tag="pt", name="pt2")
                    for u in range(2):
                        jk = 2 * pr + u
                        stp = psA if (2 * pr + u) % 3 == 2 else psC
                        st = stp.tile(
                            [128, 512], f32,
                            tag="a" if stp is psA else "c", name="st",
                        )
                        for i2 in range(2):
                            nc.tensor.matmul(
                                st,
                                k_all[:, 2 * i2:2 * i2 + 2,
                                      jk * 128:(jk + 1) * 128],
                                q_all[:, 2 * i2:2 * i2 + 2, q0:q0 + 512],
                                start=(i2 == 0), stop=(i2 == 1), perf_mode=DR,
                            )
                        nc.scalar.activation(
                            pt2[:, u, :], st, AF.Exp,
                            scale=rc_q[:, jk:jk + 1], bias=expb,
                        )
                    for ci in range(PC):
                        nc.tensor.matmul(
                            avps[ci],
                            v_all[:, 2 * pr:2 * pr + 2, ci * 128:(ci + 1) * 128],
                            pt2,
                            start=(pr == 0), stop=(pr == 15), perf_mode=DR,
                        )
                    nc.tensor.matmul(
                        sp, ones2, pt2,
                        start=(pr == 0), stop=(pr == 15), perf_mode=DR,
                    )
                # softmax normalize + project + residual
                # (every partition of sp carries the same sumexp row)
                rcb = small.tile([128, 512], f32, tag="rcb", name=f"rcb{g}")
                nc.vector.reciprocal_approx_fast(out=rcb, in_=sp)
                avn = avn_pool.tile([128, PC, 512], f8, name="avn")
                for ci in range(PC):
                    nc.vector.tensor_mul(avn[:, ci, :], avps[ci], rcb)
                out_t = out_pool.tile([128, PC, 512], f32, name="outt")
                for co in range(PC):
                    pop = psC.tile([128, 512], f32, tag="c", name="pop")
                    for i2 in range(2):
                        nc.tensor.matmul(
                            pop,
                            wp_sb[:, 2 * i2:2 * i2 + 2, co * 128:(co + 1) * 128],
                            avn[:, 2 * i2:2 * i2 + 2, :],
                            start=(i2 == 0), stop=(i2 == 1), perf_mode=DR,
                        )
                    nc.vector.tensor_scalar(
                        out=out_t[:, co, :], in0=pop,
                        scalar1=1.0 / SQC, scalar2=bp_sb[:, co:co + 1],
                        op0=ALU.mult, op1=ALU.add,
                    )
                    for h in range(2):
                        eng = nc.gpsimd if h == 0 else nc.vector
                        eng.tensor_add(
                            out_t[:, co, h * 256:(h + 1) * 256],
                            out_t[:, co, h * 256:(h + 1) * 256],
                            x_sb[:, co, q0 + h * 256:q0 + (h + 1) * 256],
                        )
                        (nc.sync if h == 0 else nc.scalar).dma_start(
                            out=outr[:, co, q0 + h * 256:q0 + (h + 1) * 256],
                            in_=out_t[:, co, h * 256:(h + 1) * 256],
                        )

            # ---- emission ----
            nc.sync.dma_start(out=x8[:, :, 0:CH], in_=x8d[0])
            nc.scalar.dma_start(out=wk_sb, in_=wkd[:])
            nc.gpsimd.dma_start(out=wv_sb, in_=wvd[:])
            nc.sync.dma_start(out=x8[:, :, CH:2 * CH], in_=x8d[1])
            nc.scalar.dma_start(out=wq_sb, in_=wqd[:])
            nc.scalar.dma_start(out=bq_sb, in_=bqd[:])
            nc.sync.dma_start(out=x8[:, :, 2 * CH:3 * CH], in_=x8d[2])

            stg = {}
            for j in range(NJ):
                if j + 3 < NJ:
                    nc.sync.dma_start(
                        out=x8[:, :, (j + 3) * CH:(j + 4) * CH],
                        in_=x8d[j + 3],
                    )
                # residual x chunks trickle in; only needed at the end
                x8q = x2p.tile([128, PC, CH], f8, name="x8q", tag="x8q")
                nc.gpsimd.dma_start(out=x8q, in_=xqd[j])
                nc.sync.dma_start(
                    out=x_sb[:, :, j * CH:(j + 1) * CH], in_=xd[j]
                )
                qb = j // 2
                if j % 2 == 0:
                    stg[qb] = (
                        stage.tile([4, 256], f32, name="stgs", tag="stgs"),
                        stage.tile([4, 256], f32, name="stgq", tag="stgq"),
                    )
                srow = stage.tile([1, 1024], f32, name="sr", tag="sr")
                stats_chunk(j, srow, stg[qb][0], stg[qb][1], x8q)
                k_chunk(j)
                if j % 2 == 1:
                    rchain(qb, stg[qb][0], stg[qb][1])
                if j >= 3:
                    v_chunk(j - 3)
                if j - 3 in (0, 1):
                    q_chunk(j - 3)
                if j == NJ - 1:
                    nc.sync.dma_start(out=wp_sb, in_=wpd[:])
                    nc.scalar.dma_start(out=bp_sb, in_=bpd[:])
            for j in range(5, 8):
                v_chunk(j)
            attention_group(0)
            attention_group(1)

    nc.compile()
    return nc


def _get_nc():
    global _cached_nc
    if _cached_nc is None:
        _cached_nc = _build_nc()
    return _cached_nc


def kernel(x, norm_w, w_qkv, b_qkv, w_proj, b_proj):
    import ml_dtypes

    f8np = ml_dtypes.float8_e4m3

    x = np.asarray(x, dtype=np.float32)
    norm_w = np.asarray(norm_w, dtype=np.float32)
    w_qkv = np.asarray(w_qkv, dtype=np.float32)
    b_qkv = np.asarray(b_qkv, dtype=np.float32)
    w_proj = np.asarray(w_proj, dtype=np.float32)
    b_proj = np.asarray(b_proj, dtype=np.float32)

    B = x.shape[0]

    # fold norm_w + LN centering into the QKV weights; sqrt(C) into all
    Wq = w_qkv[0:C] * norm_w[None, :]
    Wk = w_qkv[C:2 * C] * norm_w[None, :]
    Wv = w_qkv[2 * C:3 * C] * norm_w[None, :]

    def wtile(wt):  # [cin, cout] -> [128, PC, cout]
        return np.ascontiguousarray(
            wt.reshape(PC, 128, C).transpose(1, 0, 2).astype(f8np)
        )

    Wqt = wtile(((Wq - Wq.mean(1, keepdims=True)) * SQC).T)
    Wkt = wtile(((Wk - Wk.mean(1, keepdims=True)) * SQC).T)
    Wvt = wtile(((Wv - Wv.mean(1, keepdims=True)) * SQC).T)
    Wpt = wtile(w_proj.T * SQC)

    def cols(b):  # [C] -> [128, 4] chunk-column layout
        return np.ascontiguousarray(b.reshape(PC, 128).T)

    bq = cols(b_qkv[0:C] * (Q8S / SQC))
    bv = b_qkv[2 * C:3 * C]
    bpt = cols(b_proj + w_proj @ bv)

    in_maps = []
    for core in range(8):
        bi, qi = core // 4, core % 4
        xl = np.roll(x[bi].reshape(C, N), -qi * NQ, axis=1)
        # pre-tile to the on-chip layout: [chunk, partition, c-chunk, col]
        xl = np.ascontiguousarray(
            xl.reshape(PC, 128, NJ, CH).transpose(2, 1, 0, 3)
        )
        x8l = xl.astype(f8np)
        x8sq = (x8l.astype(np.float32) ** 2).astype(f8np)
        in_maps.append({
            "x": xl, "x8": x8l, "xq8": x8sq,
            "wq": Wqt, "wk": Wkt, "wv": Wvt, "wp": Wpt,
            "bq": bq, "bp": bpt,
        })

    from concourse.bass_utils import run_bass_kernel_spmd

    nc = _get_nc()
    res = run_bass_kernel_spmd(nc, in_maps, core_ids=list(range(8)))

    out = np.empty((B, C, N), dtype=np.float32)
    for core in range(8):
        bi, qi = core // 4, core % 4
        out[bi][:, qi * NQ:(qi + 1) * NQ] = res.results[core]["out"]
    return out.reshape(x.shape)
